# revision 15
# baseline (speedup 1.0000x reference)
"""Trainium2 Bass kernel for nn_ConditionalAttentionLayer.

Row-sharded across 8 NeuronCores: core c computes output rows
[c*512, (c+1)*512).  Key math identity used on device:

    exp(leaky_relu(s)) = max(exp(s), exp(0.2*s)),  s = e_src[i] + e_dst[j]

and exp(s_ij) factors rank-1: exp(e_src[i]) * exp(e_dst[j]).  So the
device never evaluates transcendentals over the NxN score matrix; it
builds P^T[j,i] = adjT * max(u_j*v_i, p_j*q_i) with three elementwise
passes (ACT + 2 DVE) per tile and feeds it straight into the
attention-value matmul (with a ones-column in h for the softmax
denominator).  Host only does O(N*d) prep: h = FiLM(x@W), e-vectors,
their exps, and slicing adj rows per core.
"""

import sys
from contextlib import ExitStack

import numpy as np
import ml_dtypes

sys.path.insert(0, "/opt/trn_rl_repo")

import concourse.bass as bass  # noqa: E402
import concourse.bacc as bacc  # noqa: E402
import concourse.tile as tile  # noqa: E402
import concourse.mybir as mybir  # noqa: E402
from concourse import bass_utils  # noqa: E402
from concourse.masks import make_identity  # noqa: E402

N = 4096
INS = 256
OUTS = 64
M = 4
NCORES = 8
ROWS = N // NCORES      # 512 output rows per core
JB = N // 128           # 32 j-blocks
IT = ROWS // 128        # 4 i-tiles per core
CG = 4                  # column groups for the adj cast-load
JPG = JB // CG          # j-blocks per column group (8)
LEAK = 0.2
USCALE = 0.25           # host pre-scales u,p so mx < 1 -> mask via min(mx, adjT)

F32 = mybir.dt.float32
BF16 = mybir.dt.bfloat16
I32 = mybir.dt.int32
Alu = mybir.AluOpType
Act = mybir.ActivationFunctionType


def _trace_kernel(tc, out_d, adj_d, h_d, vq_d, up_d):
    nc = tc.nc
    with ExitStack() as ctx:
        const = ctx.enter_context(tc.tile_pool(name="const", bufs=1))
        adjt_p = ctx.enter_context(tc.tile_pool(name="adjt", bufs=4))
        work = ctx.enter_context(tc.tile_pool(name="work", bufs=4))
        accp = ctx.enter_context(tc.tile_pool(name="acc", bufs=1, space="PSUM"))
        tpp = ctx.enter_context(tc.tile_pool(name="tp", bufs=2, space="PSUM"))
        fin = ctx.enter_context(tc.tile_pool(name="fin", bufs=2))

        # ---- persistent loads ----
        # h_aug [4096, 260] -> [128, 32, 260]
        h_sb = const.tile([128, JB, M * 65], BF16, tag="h")
        nc.sync.dma_start(h_sb, h_d.rearrange("(t p) f -> p t f", p=128))

        # broadcast exp(e_src) tiles: [M, 2, 128, ROWS]
        vq_sb = const.tile([128, M, 2, ROWS], BF16, tag="vq")
        nc.sync.dma_start(vq_sb, vq_d.rearrange("m s p i -> p m s i"))

        # per-j-block scalar columns: [M, 2, 128, JB] fp32
        up_sb = const.tile([128, M, 2, JB], F32, tag="up")
        nc.sync.dma_start(up_sb, up_d.rearrange("m s p t -> p m s t"))

        ident = const.tile([128, 128], F32, tag="ident")
        make_identity(nc, ident)

        # ---- adj load (cast int32 -> bf16 during DMA) ----
        # column groups of 512 j's so transposes unblock early
        adj_bf = []
        adj_r = adj_d.rearrange("(t p) (g j) -> g p t j", p=128, g=CG)
        for g in range(CG):
            t_ = const.tile([128, IT, N // CG], BF16, tag=f"adjbf{g}")
            nc.gpsimd.dma_start(t_, adj_r[g])
            adj_bf.append(t_)

        # ---- psum accumulators: one [65, ROWS] bank per mechanism ----
        acc = [accp.tile([65, ROWS], F32, tag=f"acc{m}", name=f"acc{m}") for m in range(M)]

        # ---- heavy loop ----
        for jb in range(JB):
            g, lb = jb // JPG, jb % JPG
            at = adjt_p.tile([128, ROWS], BF16, tag="adjT")
            for t in range(IT):
                nc.sync.dma_start(
                    at[:, t * 128:(t + 1) * 128],
                    adj_bf[g][:, t, lb * 128:(lb + 1) * 128],
                    transpose=True,
                )
            for m in range(M):
                k = jb * M + m
                # m2 = p_j * q_bcast — ACT mostly, DVE ts for 1 in 5
                m2 = work.tile([128, ROWS], BF16, tag="m2")
                nc.scalar.activation(
                    m2, vq_sb[:, m, 1, :], Act.Copy,
                    scale=up_sb[:, m, 1, jb:jb + 1],
                )
                # m1 = u_j * v_bcast  (DVE ts, 4x mode)
                m1 = work.tile([128, ROWS], BF16, tag="m1")
                nc.vector.tensor_scalar(
                    m1, vq_sb[:, m, 0, :], up_sb[:, m, 0, jb:jb + 1],
                    None, Alu.mult,
                )
                # Mx = max(m1, m2)  (DVE tt, 2x mode)
                mx = work.tile([128, ROWS], BF16, tag="mx")
                nc.vector.tensor_tensor(mx, m1, m2, Alu.max)
                # P = min(Mx, adjT)  (mx < 1 by USCALE; adjT in {0,1})
                pt = work.tile([128, ROWS], BF16, tag="pt", name="pt")
                nc.vector.tensor_tensor(pt, mx, at, Alu.min)
                # accumulate out^T[m] += h_aug[jb, m].T @ P
                nc.tensor.matmul(
                    acc[m],
                    lhsT=h_sb[:, jb, m * 65:(m + 1) * 65],
                    rhs=pt,
                    start=(jb == 0), stop=(jb == JB - 1),
                )

        # ---- epilogue: transpose, normalize, elu, store ----
        o65s = []
        for m in range(M):
            o65 = fin.tile([65, ROWS], F32, tag=f"o65_{m}", name=f"o65_{m}")
            nc.scalar.activation(o65, acc[m], Act.Copy)
            o65s.append(o65)
        out_r = out_d.rearrange("(c p) f -> c p f", p=128)
        for c in range(IT):
            ob = fin.tile([128, M * OUTS], F32, tag="ob")
            for m in range(M):
                o65 = o65s[m]
                pt_t = tpp.tile([128, 65], F32, tag="ptt")
                nc.tensor.transpose(
                    pt_t, o65[:, c * 128:(c + 1) * 128], ident[0:65, 0:65]
                )
                rcp = fin.tile([128, 1], F32, tag="rcp")
                nc.vector.reciprocal(rcp, pt_t[:, 64:65])
                xn = fin.tile([128, OUTS], F32, tag="xn")
                nc.vector.tensor_scalar(xn, pt_t[:, 0:OUTS], rcp, None, Alu.mult)
                mn = fin.tile([128, OUTS], F32, tag="mn")
                nc.vector.tensor_scalar(mn, xn, 0.0, None, Alu.min)
                eq = fin.tile([128, OUTS], F32, tag="eq")
                nc.scalar.activation(eq, mn, Act.Exp)
                nc.vector.scalar_tensor_tensor(
                    ob[:, m * OUTS:(m + 1) * OUTS], eq, -1.0, xn,
                    Alu.add, Alu.max,
                )
            nc.sync.dma_start(out_r[c], ob)


_CACHE = {}


def _build():
    if "nc" in _CACHE:
        return _CACHE["nc"]
    nc = bacc.Bacc("TRN2", target_bir_lowering=False, debug=False,
                   num_devices=NCORES)
    adj_d = nc.dram_tensor("adj_rows", [ROWS, N], I32, kind="ExternalInput").ap()
    h_d = nc.dram_tensor("h_aug", [N, M * 65], BF16, kind="ExternalInput").ap()
    vq_d = nc.dram_tensor("vq_bcast", [M, 2, 128, ROWS], BF16,
                          kind="ExternalInput").ap()
    up_d = nc.dram_tensor("up_col", [M, 2, 128, JB], F32,
                          kind="ExternalInput").ap()
    out_d = nc.dram_tensor("out", [ROWS, M * OUTS], F32,
                           kind="ExternalOutput").ap()
    with tile.TileContext(nc) as tc:
        _trace_kernel(tc, out_d, adj_d, h_d, vq_d, up_d)
    nc.compile()
    _CACHE["nc"] = nc
    return nc


def host_prep(x, adj, W, a1, a2, Wc, bc):
    x = np.asarray(x, np.float32)
    pooled = x.mean(0)
    gb = (pooled @ np.asarray(Wc, np.float32) + np.asarray(bc, np.float32))
    gb = gb.reshape(2, M, OUTS)
    gamma, beta = gb[0], gb[1]
    h = np.einsum("ni,mio->mno", x, np.asarray(W, np.float32))
    h = gamma[:, None, :] * h + beta[:, None, :]          # [M, N, OUTS]
    e_src = np.einsum("mno,mo->mn", h, np.asarray(a1, np.float32))
    e_dst = np.einsum("mno,mo->mn", h, np.asarray(a2, np.float32))

    h_aug = np.zeros((N, M * 65), np.float32)
    for m in range(M):
        h_aug[:, m * 65:m * 65 + OUTS] = h[m]
        h_aug[:, m * 65 + OUTS] = 1.0
    h_aug = h_aug.astype(ml_dtypes.bfloat16)

    u = np.exp(e_dst) * USCALE           # [M, N]; scale cancels in softmax
    p = np.exp(LEAK * e_dst) * USCALE
    v = np.exp(e_src)
    q = np.exp(LEAK * e_src)

    up_col = np.empty((M, 2, 128, JB), np.float32)
    for m in range(M):
        up_col[m, 0] = u[m].reshape(JB, 128).T
        up_col[m, 1] = p[m].reshape(JB, 128).T

    in_maps = []
    for c in range(NCORES):
        sl = slice(c * ROWS, (c + 1) * ROWS)
        vq = np.empty((M, 2, 128, ROWS), np.float32)
        for m in range(M):
            vq[m, 0] = np.broadcast_to(v[m][sl], (128, ROWS))
            vq[m, 1] = np.broadcast_to(q[m][sl], (128, ROWS))
        in_maps.append({
            "adj_rows": np.ascontiguousarray(adj[sl]).astype(np.int32),
            "h_aug": h_aug,
            "vq_bcast": vq.astype(ml_dtypes.bfloat16),
            "up_col": up_col,
        })
    return in_maps


def kernel(x, adj, W, a1, a2, Wc, bc):
    nc = _build()
    in_maps = host_prep(x, adj, W, a1, a2, Wc, bc)
    res = bass_utils.run_bass_kernel_spmd(
        nc, in_maps, core_ids=list(range(NCORES))
    )
    out = np.concatenate([res.results[c]["out"] for c in range(NCORES)], axis=0)
    return out.astype(np.float32)


# revision 19
# speedup vs baseline: 1.0803x; 1.0803x over previous
"""Trainium2 Bass kernel for nn_ConditionalAttentionLayer.

Row-sharded across 8 NeuronCores: core c computes output rows
[c*512, (c+1)*512).  Key math identity used on device:

    exp(leaky_relu(s)) = max(exp(s), exp(0.2*s)),  s = e_src[i] + e_dst[j]

and exp(s_ij) factors rank-1: exp(e_src[i]) * exp(e_dst[j]).  So the
device never evaluates transcendentals over the NxN score matrix; it
builds P^T[j,i] = adjT * max(u_j*v_i, p_j*q_i) with three elementwise
passes (ACT + 2 DVE) per tile and feeds it straight into the
attention-value matmul (with a ones-column in h for the softmax
denominator).  Host only does O(N*d) prep: h = FiLM(x@W), e-vectors,
their exps, and slicing adj rows per core.
"""

import sys
from contextlib import ExitStack

import numpy as np
import ml_dtypes

sys.path.insert(0, "/opt/trn_rl_repo")

import concourse.bass as bass  # noqa: E402
import concourse.bacc as bacc  # noqa: E402
import concourse.tile as tile  # noqa: E402
import concourse.mybir as mybir  # noqa: E402
from concourse import bass_utils  # noqa: E402
from concourse.masks import make_identity  # noqa: E402

N = 4096
INS = 256
OUTS = 64
M = 4
NCORES = 8
ROWS = N // NCORES      # 512 output rows per core
JB = N // 128           # 32 j-blocks
IT = ROWS // 128        # 4 i-tiles per core
CG = 4                  # column groups for the adj cast-load
JPG = JB // CG          # j-blocks per column group (8)
LEAK = 0.2
USCALE = 0.25           # host pre-scales u,p so mx < 1 -> mask via min(mx, adjT)

F32 = mybir.dt.float32
BF16 = mybir.dt.bfloat16
I32 = mybir.dt.int32
Alu = mybir.AluOpType
Act = mybir.ActivationFunctionType


def _trace_kernel(tc, out_d, adj_d, h_d, vq_d, up_d):
    nc = tc.nc
    with ExitStack() as ctx:
        const = ctx.enter_context(tc.tile_pool(name="const", bufs=1))
        adjt_p = ctx.enter_context(tc.tile_pool(name="adjt", bufs=4))
        work = ctx.enter_context(tc.tile_pool(name="work", bufs=4))
        accp = ctx.enter_context(tc.tile_pool(name="acc", bufs=1, space="PSUM"))
        tpp = ctx.enter_context(tc.tile_pool(name="tp", bufs=2, space="PSUM"))
        fin = ctx.enter_context(tc.tile_pool(name="fin", bufs=2))

        # ---- persistent loads ----
        # h_aug [4096, 260] -> [128, 32, 260]
        h_sb = const.tile([128, JB, M * 65], BF16, tag="h")
        nc.sync.dma_start(h_sb, h_d.rearrange("(t p) f -> p t f", p=128))

        # broadcast exp(e_src) tiles: [M, 2, 128, ROWS]
        vq_sb = const.tile([128, M, 2, ROWS], BF16, tag="vq")
        nc.sync.dma_start(vq_sb, vq_d.rearrange("m s p i -> p m s i"))

        # per-j-block scalar columns: [M, 2, 128, JB] fp32
        up_sb = const.tile([128, M, 2, JB], F32, tag="up")
        nc.sync.dma_start(up_sb, up_d.rearrange("m s p t -> p m s t"))

        ident = const.tile([128, 128], F32, tag="ident")
        make_identity(nc, ident)

        # ---- adj load (cast int32 -> bf16 during DMA) ----
        # column groups of 512 j's so transposes unblock early
        adj_bf = []
        adj_r = adj_d.rearrange("(t p) (g j) -> g p t j", p=128, g=CG)
        for g in range(CG):
            t_ = const.tile([128, IT, N // CG], BF16, tag=f"adjbf{g}")
            nc.gpsimd.dma_start(t_, adj_r[g])
            adj_bf.append(t_)

        # ---- psum accumulators: one [65, ROWS] bank per mechanism ----
        acc = [accp.tile([65, ROWS], F32, tag=f"acc{m}", name=f"acc{m}") for m in range(M)]

        # ---- heavy loop ----
        for jb in range(JB):
            g, lb = jb // JPG, jb % JPG
            at = adjt_p.tile([128, ROWS], BF16, tag="adjT")
            for t in range(IT):
                nc.sync.dma_start(
                    at[:, t * 128:(t + 1) * 128],
                    adj_bf[g][:, t, lb * 128:(lb + 1) * 128],
                    transpose=True,
                )
            # rank-1 factors for all 4 mechanisms into one [128, 2, M, ROWS]
            m12 = work.tile([128, 2, M, ROWS], BF16, tag="m12")
            for m in range(M):
                # m2 = p_j * q_bcast  (ACT Copy w/ per-partition scale)
                nc.scalar.activation(
                    m12[:, 1, m, :], vq_sb[:, m, 1, :], Act.Copy,
                    scale=up_sb[:, m, 1, jb:jb + 1],
                )
                # m1 = u_j * v_bcast  (DVE ts, 4x mode)
                nc.vector.tensor_scalar(
                    m12[:, 0, m, :], vq_sb[:, m, 0, :],
                    up_sb[:, m, 0, jb:jb + 1], None, Alu.mult,
                )
            # Mx = max(m1, m2), P = min(Mx, adjT): one 2048-wide tt each
            mx = work.tile([128, M, ROWS], BF16, tag="mx")
            nc.vector.tensor_tensor(mx, m12[:, 0], m12[:, 1], Alu.max)
            pt = work.tile([128, M, ROWS], BF16, tag="pt")
            at_b = bass.AP(at.tensor, at.offset,
                           [list(at.ap[0]), [0, M], list(at.ap[1])])
            nc.vector.tensor_tensor(pt, mx, at_b, Alu.min)
            for m in range(M):
                # accumulate out^T[m] += h_aug[jb, m].T @ P[m]
                nc.tensor.matmul(
                    acc[m],
                    lhsT=h_sb[:, jb, m * 65:(m + 1) * 65],
                    rhs=pt[:, m, :],
                    start=(jb == 0), stop=(jb == JB - 1),
                )

        # ---- epilogue: transpose, normalize, elu, store ----
        o65s = []
        for m in range(M):
            o65 = fin.tile([65, ROWS], F32, tag=f"o65_{m}", name=f"o65_{m}")
            nc.scalar.activation(o65, acc[m], Act.Copy)
            o65s.append(o65)
        out_r = out_d.rearrange("(c p) f -> c p f", p=128)
        for c in range(IT):
            ob = fin.tile([128, M * OUTS], F32, tag="ob")
            for m in range(M):
                o65 = o65s[m]
                pt_t = tpp.tile([128, 65], F32, tag="ptt")
                nc.tensor.transpose(
                    pt_t, o65[:, c * 128:(c + 1) * 128], ident[0:65, 0:65]
                )
                rcp = fin.tile([128, 1], F32, tag="rcp")
                nc.vector.reciprocal(rcp, pt_t[:, 64:65])
                xn = fin.tile([128, OUTS], F32, tag="xn")
                nc.vector.tensor_scalar(xn, pt_t[:, 0:OUTS], rcp, None, Alu.mult)
                mn = fin.tile([128, OUTS], F32, tag="mn")
                nc.vector.tensor_scalar(mn, xn, 0.0, None, Alu.min)
                eq = fin.tile([128, OUTS], F32, tag="eq")
                nc.scalar.activation(eq, mn, Act.Exp)
                nc.vector.scalar_tensor_tensor(
                    ob[:, m * OUTS:(m + 1) * OUTS], eq, -1.0, xn,
                    Alu.add, Alu.max,
                )
            nc.sync.dma_start(out_r[c], ob)


_CACHE = {}


def _build():
    if "nc" in _CACHE:
        return _CACHE["nc"]
    nc = bacc.Bacc("TRN2", target_bir_lowering=False, debug=False,
                   num_devices=NCORES)
    adj_d = nc.dram_tensor("adj_rows", [ROWS, N], I32, kind="ExternalInput").ap()
    h_d = nc.dram_tensor("h_aug", [N, M * 65], BF16, kind="ExternalInput").ap()
    vq_d = nc.dram_tensor("vq_bcast", [M, 2, 128, ROWS], BF16,
                          kind="ExternalInput").ap()
    up_d = nc.dram_tensor("up_col", [M, 2, 128, JB], F32,
                          kind="ExternalInput").ap()
    out_d = nc.dram_tensor("out", [ROWS, M * OUTS], F32,
                           kind="ExternalOutput").ap()
    with tile.TileContext(nc) as tc:
        _trace_kernel(tc, out_d, adj_d, h_d, vq_d, up_d)
    nc.compile()
    _CACHE["nc"] = nc
    return nc


def host_prep(x, adj, W, a1, a2, Wc, bc):
    x = np.asarray(x, np.float32)
    pooled = x.mean(0)
    gb = (pooled @ np.asarray(Wc, np.float32) + np.asarray(bc, np.float32))
    gb = gb.reshape(2, M, OUTS)
    gamma, beta = gb[0], gb[1]
    h = np.einsum("ni,mio->mno", x, np.asarray(W, np.float32))
    h = gamma[:, None, :] * h + beta[:, None, :]          # [M, N, OUTS]
    e_src = np.einsum("mno,mo->mn", h, np.asarray(a1, np.float32))
    e_dst = np.einsum("mno,mo->mn", h, np.asarray(a2, np.float32))

    h_aug = np.zeros((N, M * 65), np.float32)
    for m in range(M):
        h_aug[:, m * 65:m * 65 + OUTS] = h[m]
        h_aug[:, m * 65 + OUTS] = 1.0
    h_aug = h_aug.astype(ml_dtypes.bfloat16)

    u = np.exp(e_dst) * USCALE           # [M, N]; scale cancels in softmax
    p = np.exp(LEAK * e_dst) * USCALE
    v = np.exp(e_src)
    q = np.exp(LEAK * e_src)

    up_col = np.empty((M, 2, 128, JB), np.float32)
    for m in range(M):
        up_col[m, 0] = u[m].reshape(JB, 128).T
        up_col[m, 1] = p[m].reshape(JB, 128).T

    in_maps = []
    for c in range(NCORES):
        sl = slice(c * ROWS, (c + 1) * ROWS)
        vq = np.empty((M, 2, 128, ROWS), np.float32)
        for m in range(M):
            vq[m, 0] = np.broadcast_to(v[m][sl], (128, ROWS))
            vq[m, 1] = np.broadcast_to(q[m][sl], (128, ROWS))
        in_maps.append({
            "adj_rows": np.ascontiguousarray(adj[sl]).astype(np.int32),
            "h_aug": h_aug,
            "vq_bcast": vq.astype(ml_dtypes.bfloat16),
            "up_col": up_col,
        })
    return in_maps


def kernel(x, adj, W, a1, a2, Wc, bc):
    nc = _build()
    in_maps = host_prep(x, adj, W, a1, a2, Wc, bc)
    res = bass_utils.run_bass_kernel_spmd(
        nc, in_maps, core_ids=list(range(NCORES))
    )
    out = np.concatenate([res.results[c]["out"] for c in range(NCORES)], axis=0)
    return out.astype(np.float32)
